# revision 1
# baseline (speedup 1.0000x reference)
"""DAN classifier (embedding gather + mean-pool + tiny MLP + batch log-softmax)
on 8 Trainium2 NeuronCores.

Sharding: data-parallel over the batch (sentence) dim — 2048 sentences/core.
The embedding table (padded to [400000, 320] f32 so rows are 256B-aligned)
and the tiny MLP weights are replicated on every core.

Per-core device kernel (16 groups of 128 sentences, 6400 tokens each):
  - The vocab is split into 13 buckets of 32768 rows so row indices fit the
    int16 index format of InstDMAGatherAnt (dma_gather). The host buckets
    each group's tokens, pads each bucket list to a cross-core budget with
    index-0 dummies, and uploads the int16 indices (16-partition-wrapped,
    replicated to 128 partitions) plus a per-slot sentence-id table.
  - 13 dma_gather ops per group (4 SWDGE queues round-robin) pull the token
    rows into SBUF tiles laid out [128, nblk, 320]: gathered slot k ->
    partition k%128, block k//128.
  - Pooling: per 128-slot block, a one-hot matrix S[k, s] = (sent[k] == s),
    built on DVE by comparing the sentence-id table against an iota
    constant, maps slots to sentences: PE matmuls S_blk.T @ G_blk
    accumulate the token-sum into PSUM [128 sentences, 300]. Partial tail
    blocks run with K=rem so unwritten slots are never read. S matrices are
    built one group ahead so every matmul needs at most one sync wait
    (HW limit: one embedded wait per compute instruction).
  - MLP: PE transpose of pooled -> [300, 128]; matmuls against V_w.T/SEQ
    (the 1/50 mean fold), ReLU+bias on ACT, W matmul, W_b add on DVE.
  - One DMA writes logits.T [2, 2048] to DRAM.

Host glue: shard/pack tokens, run SPMD on cores 0-7, concatenate the logit
slabs and apply the global log-softmax over the batch axis (16384x2 —
negligible next to the ~1GB on-device gather).
"""

import numpy as np

VOCAB, DIM, HID, OUT = 400000, 300, 32, 2
BATCH, SEQ = 16384, 50
N_CORES = 8
B_CORE = BATCH // N_CORES            # 2048 sentences per core
GROUP = 128                          # sentences per group
N_GROUPS = B_CORE // GROUP           # 16
EPAD = 320                           # padded row length (1280 B, 256B-aligned)
BUCKET = 32768                       # int16-addressable rows per bucket
NB = -(-VOCAB // BUCKET)             # 13
DCH = (128, 128, DIM - 256)          # contraction chunks over DIM
N_QUEUES = 4


def _cdiv(a, b):
    return -(-a // b)


class _Plan:
    """Per-(group,bucket) budgets and packed-layout offsets shared by the
    host packer and the device builder."""

    def __init__(self, budgets):
        self.budgets = budgets                      # [n_groups][NB] ints
        self.icol_off = []                          # idx col offset per (g,b)
        self.blk_off = []                           # sent blk offset per (g,b)
        self.nblk_g = []                            # blocks per group
        io = 0
        bo = 0
        for g in range(len(budgets)):
            row_i, row_b = [], []
            blk0 = bo
            for b in range(NB):
                n = budgets[g][b]
                row_i.append(io)
                row_b.append(bo)
                io += _cdiv(n, 16)
                bo += _cdiv(n, 128)
            self.icol_off.append(row_i)
            self.blk_off.append(row_b)
            self.nblk_g.append(bo - blk0)
        self.icols_tot = io
        self.nblk_tot = bo
        self.max_nblk_g = max(self.nblk_g)
        self.max_nblk_b = max(_cdiv(n, 128) for row in budgets for n in row)

    def key(self):
        return tuple(tuple(r) for r in self.budgets)


def _build_bass(plan, vocab=VOCAB, dim=DIM, hid=HID, nout=OUT,
                b_core=B_CORE, group=GROUP, n_cores=N_CORES):
    from contextlib import ExitStack

    import concourse.tile as tile
    from concourse import bacc, mybir

    f32 = mybir.dt.float32
    i16 = mybir.dt.int16
    n_groups = b_core // group
    dch = DCH
    nch = len(dch)

    nc = bacc.Bacc("TRN2", target_bir_lowering=False, debug=False,
                   enable_asserts=False, num_devices=n_cores,
                   num_swdge_queues=N_QUEUES)
    t_idx = nc.declare_dram_parameter("gidx", [128, plan.icols_tot], i16,
                                      isOutput=False)
    t_sent = nc.declare_dram_parameter("sent", [128, plan.nblk_tot], f32,
                                       isOutput=False)
    t_iota = nc.declare_dram_parameter("iota", [128, group], f32,
                                       isOutput=False)
    t_ident = nc.declare_dram_parameter("ident", [128, 128], f32,
                                        isOutput=False)
    t_emb = nc.declare_dram_parameter("embp", [vocab, EPAD], f32,
                                      isOutput=False)
    t_vwt = nc.declare_dram_parameter("vwt", [128, nch * hid], f32,
                                      isOutput=False)
    t_vb = nc.declare_dram_parameter("vb", [hid, 1], f32, isOutput=False)
    t_wwt = nc.declare_dram_parameter("wwt", [hid, nout], f32, isOutput=False)
    t_wb = nc.declare_dram_parameter("wb", [nout, 1], f32, isOutput=False)
    t_out = nc.declare_dram_parameter("out", [nout, b_core], f32,
                                      isOutput=True)

    relu = mybir.ActivationFunctionType.Relu
    is_eq = mybir.AluOpType.is_equal

    with ExitStack() as ctx:
        tc = ctx.enter_context(tile.TileContext(nc))
        consts = ctx.enter_context(tc.tile_pool(name="consts", bufs=1))
        gpool = ctx.enter_context(tc.tile_pool(name="gather", bufs=12))
        spool = ctx.enter_context(tc.tile_pool(name="smat", bufs=2))
        sbp = ctx.enter_context(tc.tile_pool(name="sbwork", bufs=2))
        pp_pool = ctx.enter_context(tc.tile_pool(name="ppool", bufs=2, space="PSUM"))
        pt_pool = ctx.enter_context(tc.tile_pool(name="ptpool", bufs=2, space="PSUM"))
        ph_pool = ctx.enter_context(tc.tile_pool(name="phpool", bufs=2, space="PSUM"))
        pl_pool = ctx.enter_context(tc.tile_pool(name="plpool", bufs=1, space="PSUM"))
        pd_pool = ctx.enter_context(tc.tile_pool(name="pdpool", bufs=1, space="PSUM"))

        idx_sb = consts.tile([128, plan.icols_tot], i16)
        nc.sync.dma_start(idx_sb[:], t_idx[:])
        sent_sb = consts.tile([128, plan.nblk_tot], f32)
        nc.sync.dma_start(sent_sb[:], t_sent[:])
        iota_sb = consts.tile([128, group], f32)
        nc.sync.dma_start(iota_sb[:], t_iota[:])
        ident = consts.tile([128, 128], f32)
        nc.sync.dma_start(ident[:], t_ident[:])
        vwt_sb = consts.tile([128, nch * hid], f32)
        nc.sync.dma_start(vwt_sb[:], t_vwt[:])
        vb_sb = consts.tile([hid, 1], f32)
        nc.sync.dma_start(vb_sb[:], t_vb[:])
        wwt_sb = consts.tile([hid, nout], f32)
        nc.sync.dma_start(wwt_sb[:], t_wwt[:])
        wb_sb = consts.tile([nout, 1], f32)
        nc.sync.dma_start(wb_sb[:], t_wb[:])
        out_sb = consts.tile([nout, b_core], f32)

        # Compute instructions carry at most ONE embedded sync wait after
        # codegen. Prime each engine's vector clock on every external
        # producer it will consume mid-loop, so steady-state instructions
        # need only the wait on their data tile.
        dumb_dve = consts.tile([hid, 1], f32)
        nc.vector.tensor_copy(dumb_dve[0:nout, :], wb_sb[:])
        nc.vector.tensor_copy(dumb_dve[:], sent_sb[0:hid, 0:1])
        nc.vector.tensor_copy(dumb_dve[:], iota_sb[0:hid, 0:1])
        dumb_act = consts.tile([hid, 1], f32)
        nc.scalar.copy(dumb_act[:], vb_sb[:])
        dumb_ps = pd_pool.tile([1, 1], f32)
        nc.tensor.matmul(dumb_ps[:], lhsT=ident[:, 0:1], rhs=ident[:, 0:1],
                         start=True, stop=True)
        nc.tensor.matmul(dumb_ps[:], lhsT=vwt_sb[:, 0:1], rhs=vwt_sb[:, 0:1],
                         start=True, stop=True)
        nc.tensor.matmul(dumb_ps[:], lhsT=wwt_sb[:, 0:1], rhs=wwt_sb[:, 0:1],
                         start=True, stop=True)

        def build_s(g):
            """One-hot S for all blocks of group g: S[k, blk, s] =
            (sent[k, blk] == s), one DVE op."""
            nblk = plan.nblk_g[g]
            s_t = spool.tile([128, plan.max_nblk_g * group], f32, tag="S")
            boff = plan.blk_off[g][0]
            in0 = sent_sb[:, boff:boff + nblk].to_broadcast([128, nblk, group])
            in1 = (iota_sb[:].rearrange("p (a c) -> p a c", a=1)
                   .to_broadcast([128, nblk, group]))
            nc.vector.tensor_tensor(
                out=s_t[:, 0:nblk * group].rearrange("p (c s) -> p c s",
                                                     s=group),
                in0=in0, in1=in1, op=is_eq)
            return s_t

        s_tiles = {0: build_s(0)}
        # prime PE on the DVE-built S
        nc.tensor.matmul(dumb_ps[:], lhsT=s_tiles[0][:, 0:1],
                         rhs=s_tiles[0][:, 0:1], start=True, stop=True)

        gather_ct = 0
        for g in range(n_groups):
            gtiles = []
            for b in range(NB):
                n = plan.budgets[g][b]
                if n == 0:
                    gtiles.append(None)
                    continue
                nblk = _cdiv(n, 128)
                gt = gpool.tile([128, plan.max_nblk_b * EPAD], f32, tag="G")
                rows = min(BUCKET, vocab - b * BUCKET)
                io = plan.icol_off[g][b]
                nc.gpsimd.dma_gather(
                    out_ap=gt[:, 0:nblk * EPAD].rearrange(
                        "p (c e) -> p c e", e=EPAD),
                    in_ap=t_emb[b * BUCKET: b * BUCKET + rows, :],
                    idxs_ap=idx_sb[:, io: io + _cdiv(n, 16)],
                    num_idxs=n,
                    num_idxs_reg=n,
                    elem_size=EPAD,
                    queue_num=gather_ct % N_QUEUES,
                )
                gather_ct += 1
                gtiles.append(gt)

            s_g = s_tiles.pop(g)
            # S for group g+1 built now (DVE order: before this group's
            # pooled/pt copies) so next group's matmuls carry no DVE wait.
            if g + 1 < n_groups:
                s_tiles[g + 1] = build_s(g + 1)

            pooled_ps = pp_pool.tile([group, dim], f32, tag="pooled")
            mm = 0
            n_mm = plan.nblk_g[g]
            for b in range(NB):
                n = plan.budgets[g][b]
                if n == 0:
                    continue
                nblk = _cdiv(n, 128)
                gt = gtiles[b]
                sblk0 = plan.blk_off[g][b] - plan.blk_off[g][0]
                for blk in range(nblk):
                    k = min(128, n - blk * 128)
                    nc.tensor.matmul(
                        pooled_ps[:],
                        lhsT=s_g[0:k, (sblk0 + blk) * group:
                                 (sblk0 + blk + 1) * group],
                        rhs=gt[0:k, blk * EPAD: blk * EPAD + dim],
                        start=(mm == 0),
                        stop=(mm == n_mm - 1),
                    )
                    mm += 1

            pooled_sb = sbp.tile([group, dim], f32, tag="pooled_sb")
            nc.vector.tensor_copy(pooled_sb[:], pooled_ps[:])

            pt_ps = pt_pool.tile([128, nch * group], f32, tag="pt")
            for c, w in enumerate(dch):
                nc.tensor.transpose(
                    out=pt_ps[0:w, c * group: (c + 1) * group],
                    in_=pooled_sb[:, c * 128: c * 128 + w],
                    identity=ident[:group, :group],
                )
            pt_sb = sbp.tile([128, nch * group], f32, tag="pt_sb")
            nc.vector.tensor_copy(pt_sb[:, 0:2 * group], pt_ps[:, 0:2 * group])
            nc.vector.tensor_copy(pt_sb[0:dch[2], 2 * group:3 * group],
                                  pt_ps[0:dch[2], 2 * group:3 * group])

            h_ps = ph_pool.tile([hid, group], f32, tag="h")
            for c, w in enumerate(dch):
                nc.tensor.matmul(
                    h_ps[:],
                    lhsT=vwt_sb[0:w, c * hid: (c + 1) * hid],
                    rhs=pt_sb[0:w, c * group: (c + 1) * group],
                    start=(c == 0),
                    stop=(c == nch - 1),
                )
            h_sb = sbp.tile([hid, group], f32, tag="h_sb")
            nc.scalar.activation(h_sb[:], h_ps[:], relu, bias=vb_sb[:, 0:1])

            l_ps = pl_pool.tile([nout, group], f32, tag="l")
            nc.tensor.matmul(l_ps[:], lhsT=wwt_sb[:], rhs=h_sb[:],
                             start=True, stop=True)
            nc.vector.tensor_tensor(
                out=out_sb[:, g * group: (g + 1) * group],
                in0=l_ps[:],
                in1=wb_sb[:, 0:1].to_broadcast([nout, group]),
                op=mybir.AluOpType.add,
            )

        nc.sync.dma_start(t_out[:], out_sb[:])
    nc.finalize()
    return nc


def _pack_weights(V_w, V_b, W_w, W_b, dim=DIM, hid=HID, nout=OUT, seq=SEQ):
    nch = len(DCH)
    vwt = (np.asarray(V_w, np.float32).T / np.float32(seq)).astype(np.float32)
    vwt_packed = np.zeros((128, nch * hid), np.float32)
    off = 0
    for c, w in enumerate(DCH):
        vwt_packed[0:w, c * hid: (c + 1) * hid] = vwt[off: off + w]
        off += w
    wwt = np.ascontiguousarray(np.asarray(W_w, np.float32).T)
    vb = np.asarray(V_b, np.float32).reshape(hid, 1)
    wb = np.asarray(W_b, np.float32).reshape(nout, 1)
    return vwt_packed, vb, wwt, wb


def _plan_and_pack(tokens, b_core=B_CORE, group=GROUP, seq=SEQ):
    """Bucket every core's tokens; compute cross-core budgets; pack int16
    index and sentence-id tables per core."""
    n_cores = tokens.shape[0] // b_core
    n_groups = b_core // group
    toks = np.asarray(tokens, np.int64).reshape(n_cores, n_groups, group, seq)

    # per (core, group): stable-sort tokens by bucket
    flat = toks.reshape(n_cores, n_groups, group * seq)
    sent_of = np.broadcast_to(np.arange(group)[:, None],
                              (group, seq)).reshape(group * seq)
    buck = flat >> 15
    counts = np.zeros((n_cores, n_groups, NB), np.int64)
    for b in range(NB):
        counts[:, :, b] = (buck == b).sum(axis=2)
    budgets = counts.max(axis=0)                     # [n_groups, NB]
    plan = _Plan(budgets.tolist())

    gidx = np.zeros((n_cores, 128, plan.icols_tot), np.int16)
    sent = np.full((n_cores, 128, plan.nblk_tot), -1.0, np.float32)
    for c in range(n_cores):
        for g in range(n_groups):
            order = np.argsort(buck[c, g], kind="stable")
            stoks = flat[c, g][order]
            ssent = sent_of[order]
            pos = 0
            for b in range(NB):
                n = int(counts[c, g, b])
                bud = int(budgets[g, b])
                if bud == 0:
                    continue
                loc = np.zeros(bud, np.int16)
                sen = np.full(bud, -1.0, np.float32)
                loc[:n] = (stoks[pos:pos + n] & 32767).astype(np.int16)
                sen[:n] = ssent[pos:pos + n]
                pos += n
                # wrap idx: slot i -> [i % 16, io + i // 16]
                cols = _cdiv(bud, 16)
                w = np.zeros(cols * 16, np.int16)
                w[:bud] = loc
                io = plan.icol_off[g][b]
                gidx[c, :, io:io + cols] = np.tile(
                    w.reshape(cols, 16).T, (8, 1))
                # sent: slot k -> [k % 128, bo + k // 128]
                nblk = _cdiv(bud, 128)
                sw = np.full(nblk * 128, -1.0, np.float32)
                sw[:bud] = sen
                bo = plan.blk_off[g][b]
                sent[c, :, bo:bo + nblk] = sw.reshape(nblk, 128).T
    return plan, gidx, sent


_STATE = {}


def kernel(tokens, emb, V_w, V_b, W_w, W_b, _trace=False):
    from concourse.bass_utils import run_bass_kernel_spmd

    tokens = np.asarray(tokens)
    emb = np.asarray(emb, np.float32)

    plan, gidx, sent = _plan_and_pack(tokens)
    vwt_packed, vb, wwt, wb = _pack_weights(V_w, V_b, W_w, W_b)

    embp = _STATE.get("embp")
    if embp is None or _STATE.get("embp_src") is not emb:
        embp = np.zeros((VOCAB, EPAD), np.float32)
        embp[:, :DIM] = emb
        _STATE["embp"] = embp
        _STATE["embp_src"] = emb

    iota = np.broadcast_to(np.arange(GROUP, dtype=np.float32),
                           (128, GROUP)).copy()
    ident = np.eye(128, dtype=np.float32)

    nc = None
    if _STATE.get("plan_key") == plan.key():
        nc = _STATE.get("nc")
    if nc is None:
        nc = _build_bass(plan)
        _STATE["nc"] = nc
        _STATE["plan_key"] = plan.key()

    in_maps = [
        {
            "gidx": np.ascontiguousarray(gidx[c]),
            "sent": np.ascontiguousarray(sent[c]),
            "iota": iota,
            "ident": ident,
            "embp": embp,
            "vwt": vwt_packed,
            "vb": vb,
            "wwt": wwt,
            "wb": wb,
        }
        for c in range(N_CORES)
    ]
    res = run_bass_kernel_spmd(nc, in_maps, core_ids=list(range(N_CORES)),
                               trace=_trace)
    _STATE["last_result"] = res

    logits = np.concatenate([r["out"].T for r in res.results], axis=0)

    # global log-softmax over the batch axis (LogSoftmax(dim=0))
    x = logits.astype(np.float64)
    m = x.max(axis=0, keepdims=True)
    lse = m + np.log(np.sum(np.exp(x - m), axis=0, keepdims=True))
    return (x - lse).astype(np.float32)



# revision 2
# speedup vs baseline: 1.4177x; 1.4177x over previous
"""DAN classifier (embedding gather + mean-pool + tiny MLP + batch log-softmax)
on 8 Trainium2 NeuronCores.

Sharding: data-parallel over the batch (sentence) dim — 2048 sentences/core.
The embedding table is replicated on every core as fp8-e4m3 scaled by 64
(padded to [400000, 512] so rows are 512 B — dma_gather needs elem_size
divisible by 256 B); the descale (and the 1/50 mean) folds into V_w. End to
end the fp8 quantization costs ~3e-5 rel err vs the 2e-2 budget: the
LogSoftmax(dim=0) output is dominated by -log(sum exp) over 16384 logits
of tiny variance, so per-logit absolute errors are ~1e-4.

Per-core device kernel (16 groups of 128 sentences, 6400 tokens each):
  - The vocab is split into 13 buckets of 32768 rows so row indices fit the
    int16 index format of InstDMAGatherAnt (dma_gather). The host buckets
    each group's tokens, pads each bucket list to a cross-core budget with
    index-0 dummies, and uploads the int16 indices (16-partition-wrapped,
    replicated to 128 partitions) plus a per-slot sentence-id table (bf16).
  - 13 dma_gather ops per group (4 SWDGE queues round-robin) pull the token
    rows into SBUF tiles laid out [128, nblk, 512] fp8: gathered slot k ->
    partition k%128, block k//128.
  - Pooling: per 128-slot block, a one-hot matrix S[k, s] = (sent[k] == s)
    in bf16, built on DVE by comparing the sentence-id table against an
    iota constant, maps slots to sentences: PE matmuls S_blk.T(bf16) @
    G_blk(fp8) accumulate the token-sum into PSUM [128 sentences, 300]
    fp32. bf16/fp8 matmuls run single-pass (fp32 ran LOW_HIGH dual-pass)
    and keep the PE HAM-warm at 2.4 GHz. Partial tail blocks run with
    K=rem so unwritten slots are never read; pad slots carry sent=-1 so
    their S column is all-zero and the gathered dummy rows cancel. S
    matrices are built one group ahead so every matmul needs at most one
    sync wait (HW limit: one embedded wait per compute instruction).
  - MLP: PE transpose of pooled -> [300, 128]; matmuls against
    V_w.T/(SEQ*EMB_SCALE), ReLU+bias on ACT, W matmul, W_b add on DVE.
  - One DMA writes logits.T [2, 2048] to DRAM.

Host glue: shard/pack tokens, run SPMD on cores 0-7, concatenate the logit
slabs and apply the global log-softmax over the batch axis (16384x2 —
negligible next to the ~0.4GB on-device gather).
"""

import numpy as np

VOCAB, DIM, HID, OUT = 400000, 300, 32, 2
BATCH, SEQ = 16384, 50
N_CORES = 8
B_CORE = BATCH // N_CORES            # 2048 sentences per core
GROUP = 128                          # sentences per group
N_GROUPS = B_CORE // GROUP           # 16
EPAD = 512                           # padded fp8 row length (512 B)
EMB_SCALE = 64.0                     # fp8 table stores emb * 64
BUCKET = 32768                       # int16-addressable rows per bucket
NB = -(-VOCAB // BUCKET)             # 13
DCH = (128, 128, DIM - 256)          # contraction chunks over DIM
N_QUEUES = 4


def _cdiv(a, b):
    return -(-a // b)


class _Plan:
    """Per-(group,bucket) budgets and packed-layout offsets shared by the
    host packer and the device builder."""

    def __init__(self, budgets):
        self.budgets = budgets                      # [n_groups][NB] ints
        self.icol_off = []                          # idx col offset per (g,b)
        self.blk_off = []                           # sent blk offset per (g,b)
        self.nblk_g = []                            # blocks per group
        io = 0
        bo = 0
        for g in range(len(budgets)):
            row_i, row_b = [], []
            blk0 = bo
            for b in range(NB):
                n = budgets[g][b]
                row_i.append(io)
                row_b.append(bo)
                io += _cdiv(n, 16)
                bo += _cdiv(n, 128)
            self.icol_off.append(row_i)
            self.blk_off.append(row_b)
            self.nblk_g.append(bo - blk0)
        self.icols_tot = io
        self.nblk_tot = bo
        self.max_nblk_g = max(self.nblk_g)
        self.max_nblk_b = max(_cdiv(n, 128) for row in budgets for n in row)

    def key(self):
        return tuple(tuple(r) for r in self.budgets)


def _build_bass(plan, vocab=VOCAB, dim=DIM, hid=HID, nout=OUT,
                b_core=B_CORE, group=GROUP, n_cores=N_CORES):
    from contextlib import ExitStack

    import concourse.tile as tile
    from concourse import bacc, mybir

    f32 = mybir.dt.float32
    bf16 = mybir.dt.bfloat16
    fp8 = mybir.dt.float8e4
    i16 = mybir.dt.int16
    n_groups = b_core // group
    dch = DCH
    nch = len(dch)

    nc = bacc.Bacc("TRN2", target_bir_lowering=False, debug=False,
                   enable_asserts=False, num_devices=n_cores,
                   num_swdge_queues=N_QUEUES)
    t_idx = nc.declare_dram_parameter("gidx", [128, plan.icols_tot], i16,
                                      isOutput=False)
    t_sent = nc.declare_dram_parameter("sent", [128, plan.nblk_tot], bf16,
                                       isOutput=False)
    t_iota = nc.declare_dram_parameter("iota", [128, group], bf16,
                                       isOutput=False)
    t_ident = nc.declare_dram_parameter("ident", [128, 128], f32,
                                        isOutput=False)
    t_emb = nc.declare_dram_parameter("embp", [vocab, EPAD], fp8,
                                      isOutput=False)
    t_vwt = nc.declare_dram_parameter("vwt", [128, nch * hid], f32,
                                      isOutput=False)
    t_vb = nc.declare_dram_parameter("vb", [hid, 1], f32, isOutput=False)
    t_wwt = nc.declare_dram_parameter("wwt", [hid, nout], f32, isOutput=False)
    t_wb = nc.declare_dram_parameter("wb", [nout, 1], f32, isOutput=False)
    t_out = nc.declare_dram_parameter("out", [nout, b_core], f32,
                                      isOutput=True)

    relu = mybir.ActivationFunctionType.Relu
    is_eq = mybir.AluOpType.is_equal

    with ExitStack() as ctx:
        tc = ctx.enter_context(tile.TileContext(nc))
        consts = ctx.enter_context(tc.tile_pool(name="consts", bufs=1))
        gpool = ctx.enter_context(tc.tile_pool(name="gather", bufs=26))
        spool = ctx.enter_context(tc.tile_pool(name="smat", bufs=2))
        sbp = ctx.enter_context(tc.tile_pool(name="sbwork", bufs=2))
        pp_pool = ctx.enter_context(tc.tile_pool(name="ppool", bufs=2, space="PSUM"))
        pt_pool = ctx.enter_context(tc.tile_pool(name="ptpool", bufs=2, space="PSUM"))
        ph_pool = ctx.enter_context(tc.tile_pool(name="phpool", bufs=2, space="PSUM"))
        pl_pool = ctx.enter_context(tc.tile_pool(name="plpool", bufs=1, space="PSUM"))
        pd_pool = ctx.enter_context(tc.tile_pool(name="pdpool", bufs=1, space="PSUM"))

        idx_sb = consts.tile([128, plan.icols_tot], i16)
        nc.sync.dma_start(idx_sb[:], t_idx[:])
        sent_sb = consts.tile([128, plan.nblk_tot], bf16)
        nc.sync.dma_start(sent_sb[:], t_sent[:])
        iota_sb = consts.tile([128, group], bf16)
        nc.sync.dma_start(iota_sb[:], t_iota[:])
        ident = consts.tile([128, 128], f32)
        nc.sync.dma_start(ident[:], t_ident[:])
        vwt_sb = consts.tile([128, nch * hid], f32)
        nc.sync.dma_start(vwt_sb[:], t_vwt[:])
        vb_sb = consts.tile([hid, 1], f32)
        nc.sync.dma_start(vb_sb[:], t_vb[:])
        wwt_sb = consts.tile([hid, nout], f32)
        nc.sync.dma_start(wwt_sb[:], t_wwt[:])
        wb_sb = consts.tile([nout, 1], f32)
        nc.sync.dma_start(wb_sb[:], t_wb[:])
        out_sb = consts.tile([nout, b_core], f32)

        # Compute instructions carry at most ONE embedded sync wait after
        # codegen. Prime each engine's vector clock on every external
        # producer it will consume mid-loop, so steady-state instructions
        # need only the wait on their data tile.
        dumb_dve = consts.tile([hid, 1], f32)
        nc.vector.tensor_copy(dumb_dve[0:nout, :], wb_sb[:])
        nc.vector.tensor_copy(dumb_dve[:], sent_sb[0:hid, 0:1])
        nc.vector.tensor_copy(dumb_dve[:], iota_sb[0:hid, 0:1])
        dumb_act = consts.tile([hid, 1], f32)
        nc.scalar.copy(dumb_act[:], vb_sb[:])
        dumb_ps = pd_pool.tile([1, 1], f32)
        nc.tensor.matmul(dumb_ps[:], lhsT=ident[:, 0:1], rhs=ident[:, 0:1],
                         start=True, stop=True)
        nc.tensor.matmul(dumb_ps[:], lhsT=vwt_sb[:, 0:1], rhs=vwt_sb[:, 0:1],
                         start=True, stop=True)
        nc.tensor.matmul(dumb_ps[:], lhsT=wwt_sb[:, 0:1], rhs=wwt_sb[:, 0:1],
                         start=True, stop=True)

        def build_s(g):
            """One-hot S for all blocks of group g: S[k, blk, s] =
            (sent[k, blk] == s), one DVE op, bf16 in/out (2x DVE mode)."""
            nblk = plan.nblk_g[g]
            s_t = spool.tile([128, plan.max_nblk_g * group], bf16, tag="S")
            boff = plan.blk_off[g][0]
            in0 = sent_sb[:, boff:boff + nblk].to_broadcast([128, nblk, group])
            in1 = (iota_sb[:].rearrange("p (a c) -> p a c", a=1)
                   .to_broadcast([128, nblk, group]))
            nc.vector.tensor_tensor(
                out=s_t[:, 0:nblk * group].rearrange("p (c s) -> p c s",
                                                     s=group),
                in0=in0, in1=in1, op=is_eq)
            return s_t

        s_tiles = {0: build_s(0)}
        # prime PE on the DVE-built S
        nc.tensor.matmul(dumb_ps[:], lhsT=s_tiles[0][:, 0:1],
                         rhs=s_tiles[0][:, 0:1], start=True, stop=True)

        gather_ct = 0
        for g in range(n_groups):
            gtiles = []
            for b in range(NB):
                n = plan.budgets[g][b]
                if n == 0:
                    gtiles.append(None)
                    continue
                nblk = _cdiv(n, 128)
                gt = gpool.tile([128, plan.max_nblk_b * EPAD], fp8, tag="G")
                rows = min(BUCKET, vocab - b * BUCKET)
                io = plan.icol_off[g][b]
                nc.gpsimd.dma_gather(
                    out_ap=gt[:, 0:nblk * EPAD].rearrange(
                        "p (c e) -> p c e", e=EPAD),
                    in_ap=t_emb[b * BUCKET: b * BUCKET + rows, :],
                    idxs_ap=idx_sb[:, io: io + _cdiv(n, 16)],
                    num_idxs=n,
                    num_idxs_reg=n,
                    elem_size=EPAD,
                    queue_num=gather_ct % N_QUEUES,
                )
                gather_ct += 1
                gtiles.append(gt)

            s_g = s_tiles.pop(g)
            # S for group g+1 built now (DVE order: before this group's
            # pooled/pt copies) so next group's matmuls carry no DVE wait.
            if g + 1 < n_groups:
                s_tiles[g + 1] = build_s(g + 1)

            pooled_ps = pp_pool.tile([group, dim], f32, tag="pooled")
            mm = 0
            n_mm = plan.nblk_g[g]
            for b in range(NB):
                n = plan.budgets[g][b]
                if n == 0:
                    continue
                nblk = _cdiv(n, 128)
                gt = gtiles[b]
                sblk0 = plan.blk_off[g][b] - plan.blk_off[g][0]
                for blk in range(nblk):
                    k = min(128, n - blk * 128)
                    nc.tensor.matmul(
                        pooled_ps[:],
                        lhsT=s_g[0:k, (sblk0 + blk) * group:
                                 (sblk0 + blk + 1) * group],
                        rhs=gt[0:k, blk * EPAD: blk * EPAD + dim],
                        start=(mm == 0),
                        stop=(mm == n_mm - 1),
                    )
                    mm += 1

            pooled_sb = sbp.tile([group, dim], f32, tag="pooled_sb")
            nc.vector.tensor_copy(pooled_sb[:], pooled_ps[:])

            pt_ps = pt_pool.tile([128, nch * group], f32, tag="pt")
            for c, w in enumerate(dch):
                nc.tensor.transpose(
                    out=pt_ps[0:w, c * group: (c + 1) * group],
                    in_=pooled_sb[:, c * 128: c * 128 + w],
                    identity=ident[:group, :group],
                )
            pt_sb = sbp.tile([128, nch * group], f32, tag="pt_sb")
            nc.vector.tensor_copy(pt_sb[:, 0:2 * group], pt_ps[:, 0:2 * group])
            nc.vector.tensor_copy(pt_sb[0:dch[2], 2 * group:3 * group],
                                  pt_ps[0:dch[2], 2 * group:3 * group])

            h_ps = ph_pool.tile([hid, group], f32, tag="h")
            for c, w in enumerate(dch):
                nc.tensor.matmul(
                    h_ps[:],
                    lhsT=vwt_sb[0:w, c * hid: (c + 1) * hid],
                    rhs=pt_sb[0:w, c * group: (c + 1) * group],
                    start=(c == 0),
                    stop=(c == nch - 1),
                )
            h_sb = sbp.tile([hid, group], f32, tag="h_sb")
            nc.scalar.activation(h_sb[:], h_ps[:], relu, bias=vb_sb[:, 0:1])

            l_ps = pl_pool.tile([nout, group], f32, tag="l")
            nc.tensor.matmul(l_ps[:], lhsT=wwt_sb[:], rhs=h_sb[:],
                             start=True, stop=True)
            nc.vector.tensor_tensor(
                out=out_sb[:, g * group: (g + 1) * group],
                in0=l_ps[:],
                in1=wb_sb[:, 0:1].to_broadcast([nout, group]),
                op=mybir.AluOpType.add,
            )

        nc.sync.dma_start(t_out[:], out_sb[:])
    nc.finalize()
    return nc


def _pack_weights(V_w, V_b, W_w, W_b, dim=DIM, hid=HID, nout=OUT, seq=SEQ):
    nch = len(DCH)
    vwt = (np.asarray(V_w, np.float32).T
           / np.float32(seq * EMB_SCALE)).astype(np.float32)
    vwt_packed = np.zeros((128, nch * hid), np.float32)
    off = 0
    for c, w in enumerate(DCH):
        vwt_packed[0:w, c * hid: (c + 1) * hid] = vwt[off: off + w]
        off += w
    wwt = np.ascontiguousarray(np.asarray(W_w, np.float32).T)
    vb = np.asarray(V_b, np.float32).reshape(hid, 1)
    wb = np.asarray(W_b, np.float32).reshape(nout, 1)
    return vwt_packed, vb, wwt, wb


def _plan_and_pack(tokens, b_core=B_CORE, group=GROUP, seq=SEQ):
    """Bucket every core's tokens; compute cross-core budgets; pack int16
    index and sentence-id tables per core."""
    import ml_dtypes

    n_cores = tokens.shape[0] // b_core
    n_groups = b_core // group
    toks = np.asarray(tokens, np.int64).reshape(n_cores, n_groups, group, seq)

    # per (core, group): stable-sort tokens by bucket
    flat = toks.reshape(n_cores, n_groups, group * seq)
    sent_of = np.broadcast_to(np.arange(group)[:, None],
                              (group, seq)).reshape(group * seq)
    buck = flat >> 15
    counts = np.zeros((n_cores, n_groups, NB), np.int64)
    for b in range(NB):
        counts[:, :, b] = (buck == b).sum(axis=2)
    budgets = counts.max(axis=0)                     # [n_groups, NB]
    plan = _Plan(budgets.tolist())

    gidx = np.zeros((n_cores, 128, plan.icols_tot), np.int16)
    sent = np.full((n_cores, 128, plan.nblk_tot), -1.0, ml_dtypes.bfloat16)
    for c in range(n_cores):
        for g in range(n_groups):
            order = np.argsort(buck[c, g], kind="stable")
            stoks = flat[c, g][order]
            ssent = sent_of[order]
            pos = 0
            for b in range(NB):
                n = int(counts[c, g, b])
                bud = int(budgets[g, b])
                if bud == 0:
                    continue
                loc = np.zeros(bud, np.int16)
                sen = np.full(bud, -1.0, np.float32)
                loc[:n] = (stoks[pos:pos + n] & 32767).astype(np.int16)
                sen[:n] = ssent[pos:pos + n]
                pos += n
                # wrap idx: slot i -> [i % 16, io + i // 16]
                cols = _cdiv(bud, 16)
                w = np.zeros(cols * 16, np.int16)
                w[:bud] = loc
                io = plan.icol_off[g][b]
                gidx[c, :, io:io + cols] = np.tile(
                    w.reshape(cols, 16).T, (8, 1))
                # sent: slot k -> [k % 128, bo + k // 128]
                nblk = _cdiv(bud, 128)
                sw = np.full(nblk * 128, -1.0, np.float32)
                sw[:bud] = sen
                bo = plan.blk_off[g][b]
                sent[c, :, bo:bo + nblk] = sw.reshape(nblk, 128).T.astype(
                    ml_dtypes.bfloat16)
    return plan, gidx, sent


_STATE = {}


def kernel(tokens, emb, V_w, V_b, W_w, W_b, _trace=False):
    import ml_dtypes

    from concourse.bass_utils import run_bass_kernel_spmd

    tokens = np.asarray(tokens)
    emb = np.asarray(emb, np.float32)

    plan, gidx, sent = _plan_and_pack(tokens)
    vwt_packed, vb, wwt, wb = _pack_weights(V_w, V_b, W_w, W_b)

    embp = _STATE.get("embp")
    if embp is None or _STATE.get("embp_src") is not emb:
        embp = np.zeros((VOCAB, EPAD), ml_dtypes.float8_e4m3fn)
        embp[:, :DIM] = (emb * np.float32(EMB_SCALE)).astype(
            ml_dtypes.float8_e4m3fn)
        _STATE["embp"] = embp
        _STATE["embp_src"] = emb

    iota = np.broadcast_to(np.arange(GROUP, dtype=np.float32),
                           (128, GROUP)).astype(ml_dtypes.bfloat16).copy()
    ident = np.eye(128, dtype=np.float32)

    nc = None
    if _STATE.get("plan_key") == plan.key():
        nc = _STATE.get("nc")
    if nc is None:
        nc = _build_bass(plan)
        _STATE["nc"] = nc
        _STATE["plan_key"] = plan.key()

    in_maps = [
        {
            "gidx": np.ascontiguousarray(gidx[c]),
            "sent": np.ascontiguousarray(sent[c]),
            "iota": iota,
            "ident": ident,
            "embp": embp,
            "vwt": vwt_packed,
            "vb": vb,
            "wwt": wwt,
            "wb": wb,
        }
        for c in range(N_CORES)
    ]
    res = run_bass_kernel_spmd(nc, in_maps, core_ids=list(range(N_CORES)),
                               trace=_trace)
    _STATE["last_result"] = res

    logits = np.concatenate([r["out"].T for r in res.results], axis=0)

    # global log-softmax over the batch axis (LogSoftmax(dim=0))
    x = logits.astype(np.float64)
    m = x.max(axis=0, keepdims=True)
    lse = m + np.log(np.sum(np.exp(x - m), axis=0, keepdims=True))
    return (x - lse).astype(np.float32)


# revision 6
# speedup vs baseline: 1.5332x; 1.0814x over previous
"""DAN classifier (embedding gather + mean-pool + tiny MLP + batch log-softmax)
on 8 Trainium2 NeuronCores.

Sharding: data-parallel over the batch (sentence) dim — 2048 sentences/core.
The embedding table is replicated on every core as fp8-e4m3 scaled by 64
(padded to [400000, 512] so rows are 512 B — dma_gather needs elem_size
divisible by 256 B); the descale (and the 1/50 mean) folds into V_w. End to
end the fp8 quantization costs ~3e-5 rel err vs the 2e-2 budget: the
LogSoftmax(dim=0) output is dominated by -log(sum exp) over 16384 logits of
tiny variance, so per-logit absolute errors are ~1e-4.

Per-core device kernel, 4 supergroups (SG) of 512 sentences each:
  - The vocab is split into 13 buckets of 32768 rows so row indices fit the
    int16 index format of InstDMAGatherAnt. Tokens are bucketed per
    (SG, bucket) — 52 gathers of ~2100 rows each instead of 208 small ones:
    SWDGE descriptor generation is serialized on GPSIMD Q7 cores 0-1
    across ALL queues, so per-gather fixed cost (~1 us) must be amortized.
    Slot lists are packed (no per-group alignment), sorted by (bucket,
    group), padded to a cross-core budget with index-0 dummies (~3%).
  - Gathered rows land in SBUF tiles [128, nblk, 512] fp8: slot k ->
    partition k%128, block k//128.
  - Pooling: one matmul chunk per (128-slot block, target group). Blocks
    that straddle a group boundary (union over all 8 cores, computed on
    host) get one chunk per group they touch. The one-hot S for chunk c is
    S[p, c*128+s] = (sent_chunk[p, c] == s) where the host bakes group
    masking into sent_chunk: the local sentence id if slot p of the block
    belongs to chunk c's target group, else -1. One DVE is_equal per SG
    builds all chunks; S is stored fp8 (exact 0/1) to halve SBUF and let
    the PE run fp8 x fp8. Chunks accumulate into 4 live PSUM pooled banks
    (one per group) with start/stop on the first/last chunk per group.
    fp8 matmuls run single-pass (fp32 ran LOW_HIGH dual-pass) and keep the
    PE HAM-warm at 2.4 GHz.
  - MLP per group: PE transpose of pooled -> [300, 128]; matmuls against
    V_w.T/(SEQ*EMB_SCALE), ReLU+bias on ACT, W matmul, W_b add on DVE.
  - One DMA writes logits.T [2, 2048] to DRAM.

Host glue: shard/pack tokens, run SPMD on cores 0-7, concatenate the logit
slabs and apply the global log-softmax over the batch axis (16384x2 —
negligible next to the ~0.4GB on-device gather).
"""

import numpy as np

VOCAB, DIM, HID, OUT = 400000, 300, 32, 2
BATCH, SEQ = 16384, 50
N_CORES = 8
B_CORE = BATCH // N_CORES            # 2048 sentences per core
GROUP = 128                          # sentences per group
N_GROUPS = B_CORE // GROUP           # 16
SGG = 4                              # groups per supergroup
N_SG = N_GROUPS // SGG               # 4
EPAD = 512                           # padded fp8 row length (512 B)
EMB_SCALE = 64.0                     # fp8 table stores emb * 64
BUCKET = 32768                       # int16-addressable rows per bucket
NB = -(-VOCAB // BUCKET)             # 13
DCH = (128, 128, DIM - 256)          # contraction chunks over DIM
N_QUEUES = 4


def _cdiv(a, b):
    return -(-a // b)


class _Plan:
    """Per-(SG,bucket) budgets, chunk lists, and packed-layout offsets
    shared by the host packer and the device builder."""

    def __init__(self, budgets, chunks):
        self.budgets = budgets        # [N_SG][NB] ints
        self.chunks = chunks          # [N_SG] list of (b, blk, tgt)
        self.icol_off = []            # idx col offset per (sg,b)
        io = 0
        for sg in range(len(budgets)):
            row_i = []
            for b in range(NB):
                row_i.append(io)
                io += _cdiv(budgets[sg][b], 16)
            self.icol_off.append(row_i)
        self.icols_tot = io
        self.nblk = [[_cdiv(n, 128) for n in row] for row in budgets]
        self.max_nblk_b = max(max(r) for r in self.nblk)
        self.nchunk_sg = [len(c) for c in chunks]
        self.chunk_off = []
        co = 0
        for sg in range(len(chunks)):
            self.chunk_off.append(co)
            co += self.nchunk_sg[sg]
        self.nchunk_tot = co
        self.max_nchunk_sg = max(self.nchunk_sg)
        # first/last chunk index per (sg, tgt) for PSUM start/stop flags
        self.first_chunk = []
        self.last_chunk = []
        for sg in range(len(chunks)):
            f, l = {}, {}
            for ci, (b, blk, tgt) in enumerate(chunks[sg]):
                f.setdefault(tgt, ci)
                l[tgt] = ci
            self.first_chunk.append(f)
            self.last_chunk.append(l)

    def key(self):
        return (tuple(tuple(r) for r in self.budgets),
                tuple(tuple(c) for c in self.chunks))


def _build_bass(plan, vocab=VOCAB, dim=DIM, hid=HID, nout=OUT,
                b_core=B_CORE, group=GROUP, n_cores=N_CORES):
    from contextlib import ExitStack

    import concourse.tile as tile
    from concourse import bacc, mybir

    f32 = mybir.dt.float32
    bf16 = mybir.dt.bfloat16
    fp8 = mybir.dt.float8e4
    i16 = mybir.dt.int16
    dch = DCH
    nch = len(dch)

    nc = bacc.Bacc("TRN2", target_bir_lowering=False, debug=False,
                   enable_asserts=False, num_devices=n_cores,
                   num_swdge_queues=N_QUEUES)
    t_idx = nc.declare_dram_parameter("gidx", [128, plan.icols_tot], i16,
                                      isOutput=False)
    t_sent = nc.declare_dram_parameter("sent", [128, plan.nchunk_tot], bf16,
                                       isOutput=False)
    t_iota = nc.declare_dram_parameter("iota", [128, group], bf16,
                                       isOutput=False)
    t_ident = nc.declare_dram_parameter("ident", [128, 128], f32,
                                        isOutput=False)
    t_emb = nc.declare_dram_parameter("embp", [vocab, EPAD], fp8,
                                      isOutput=False)
    t_vwt = nc.declare_dram_parameter("vwt", [128, nch * hid], f32,
                                      isOutput=False)
    t_vb = nc.declare_dram_parameter("vb", [hid, 1], f32, isOutput=False)
    t_wwt = nc.declare_dram_parameter("wwt", [hid, nout], f32, isOutput=False)
    t_wb = nc.declare_dram_parameter("wb", [nout, 1], f32, isOutput=False)
    t_out = nc.declare_dram_parameter("out", [nout, b_core], f32,
                                      isOutput=True)

    relu = mybir.ActivationFunctionType.Relu
    is_eq = mybir.AluOpType.is_equal

    with ExitStack() as ctx:
        tc = ctx.enter_context(tile.TileContext(nc))
        consts = ctx.enter_context(tc.tile_pool(name="consts", bufs=1))
        gpool = ctx.enter_context(tc.tile_pool(name="gather", bufs=8))
        spool = ctx.enter_context(tc.tile_pool(name="smat", bufs=2))
        sbp = ctx.enter_context(tc.tile_pool(name="sbwork", bufs=2))
        pp_pool = ctx.enter_context(tc.tile_pool(name="ppool", bufs=1,
                                                 space="PSUM"))
        pt_pool = ctx.enter_context(tc.tile_pool(name="ptpool", bufs=2,
                                                 space="PSUM"))
        ph_pool = ctx.enter_context(tc.tile_pool(name="phpool", bufs=1,
                                                 space="PSUM"))
        pl_pool = ctx.enter_context(tc.tile_pool(name="plpool", bufs=1,
                                                 space="PSUM"))

        idx_sb = consts.tile([128, plan.icols_tot], i16)
        nc.sync.dma_start(idx_sb[:], t_idx[:])
        sent_sb = consts.tile([128, plan.nchunk_tot], bf16)
        nc.sync.dma_start(sent_sb[:], t_sent[:])
        iota_sb = consts.tile([128, group], bf16)
        nc.sync.dma_start(iota_sb[:], t_iota[:])
        ident = consts.tile([128, 128], f32)
        nc.sync.dma_start(ident[:], t_ident[:])
        vwt_sb = consts.tile([128, nch * hid], f32)
        nc.sync.dma_start(vwt_sb[:], t_vwt[:])
        vb_sb = consts.tile([hid, 1], f32)
        nc.sync.dma_start(vb_sb[:], t_vb[:])
        wwt_sb = consts.tile([hid, nout], f32)
        nc.sync.dma_start(wwt_sb[:], t_wwt[:])
        wb_sb = consts.tile([nout, 1], f32)
        nc.sync.dma_start(wb_sb[:], t_wb[:])
        out_sb = consts.tile([nout, b_core], f32)

        # Compute instructions carry at most ONE embedded sync wait after
        # codegen. Prime each engine's vector clock on every external
        # producer it will consume mid-loop, so steady-state instructions
        # need only the wait on their data tile.
        dumb_dve = consts.tile([hid, 1], f32)
        nc.vector.tensor_copy(dumb_dve[0:nout, :], wb_sb[:])
        nc.vector.tensor_copy(dumb_dve[:], sent_sb[0:hid, 0:1])
        nc.vector.tensor_copy(dumb_dve[:], iota_sb[0:hid, 0:1])
        dumb_act = consts.tile([hid, 1], f32)
        nc.scalar.copy(dumb_act[:], vb_sb[:])
        dumb_ps = pl_pool.tile([nout, group], f32, tag="l")
        nc.tensor.matmul(dumb_ps[0:1, 0:1], lhsT=ident[:, 0:1],
                         rhs=ident[:, 0:1], start=True, stop=True)
        nc.tensor.matmul(dumb_ps[0:1, 0:1], lhsT=vwt_sb[:, 0:1],
                         rhs=vwt_sb[:, 0:1], start=True, stop=True)
        nc.tensor.matmul(dumb_ps[0:1, 0:1], lhsT=wwt_sb[:, 0:1],
                         rhs=wwt_sb[:, 0:1], start=True, stop=True)

        def build_s(sg):
            """One-hot S for all chunks of supergroup sg:
            S[p, c*128+s] = (sent_chunk[p, c] == s), one DVE op,
            fp8 out (exact 0/1)."""
            nchunk = plan.nchunk_sg[sg]
            s_t = spool.tile([128, plan.max_nchunk_sg * group], fp8, tag="S")
            coff = plan.chunk_off[sg]
            in0 = sent_sb[:, coff:coff + nchunk].to_broadcast(
                [128, nchunk, group])
            in1 = (iota_sb[:].rearrange("p (a c) -> p a c", a=1)
                   .to_broadcast([128, nchunk, group]))
            nc.vector.tensor_tensor(
                out=s_t[:, 0:nchunk * group].rearrange("p (c s) -> p c s",
                                                       s=group),
                in0=in0, in1=in1, op=is_eq)
            return s_t

        s_tiles = {0: build_s(0)}
        # prime PE on the DVE-built S
        nc.tensor.matmul(dumb_ps[0:1, 0:1], lhsT=s_tiles[0][:, 0:1],
                         rhs=s_tiles[0][:, 0:1], start=True, stop=True)

        gather_ct = 0
        for sg in range(N_SG):
            gtiles = []
            for b in range(NB):
                n = plan.budgets[sg][b]
                if n == 0:
                    gtiles.append(None)
                    continue
                nblk = plan.nblk[sg][b]
                gt = gpool.tile([128, plan.max_nblk_b * EPAD], fp8, tag="G")
                rows = min(BUCKET, vocab - b * BUCKET)
                io = plan.icol_off[sg][b]
                nc.gpsimd.dma_gather(
                    out_ap=gt[:, 0:nblk * EPAD].rearrange(
                        "p (c e) -> p c e", e=EPAD),
                    in_ap=t_emb[b * BUCKET: b * BUCKET + rows, :],
                    idxs_ap=idx_sb[:, io: io + _cdiv(n, 16)],
                    num_idxs=n,
                    num_idxs_reg=n,
                    elem_size=EPAD,
                    queue_num=gather_ct % N_QUEUES,
                    # >64 descriptors (1008 idxs) cannot fit one packet
                    single_packet=False,
                )
                gather_ct += 1
                gtiles.append(gt)

            s_sg = s_tiles.pop(sg)
            # S for sg+1 built now (DVE order: before this SG's pooled/pt
            # copies) so the next SG's matmuls carry no DVE wait.
            if sg + 1 < N_SG:
                s_tiles[sg + 1] = build_s(sg + 1)

            pooled = [pp_pool.tile([group, dim], f32, tag=f"pooled{g}",
                                   name=f"pooled{g}")
                      for g in range(SGG)]
            for ci, (b, blk, tgt) in enumerate(plan.chunks[sg]):
                k = min(128, plan.budgets[sg][b] - blk * 128)
                nc.tensor.matmul(
                    pooled[tgt][:],
                    lhsT=s_sg[0:k, ci * group: (ci + 1) * group],
                    rhs=gtiles[b][0:k, blk * EPAD: blk * EPAD + dim],
                    start=(ci == plan.first_chunk[sg][tgt]),
                    stop=(ci == plan.last_chunk[sg][tgt]),
                    skip_group_check=True,
                )

            for g in range(SGG):
                gg = sg * SGG + g
                pooled_sb = sbp.tile([group, dim], f32, tag="pooled_sb")
                nc.vector.tensor_copy(pooled_sb[:], pooled[g][:])

                pt_ps = pt_pool.tile([128, nch * group], f32, tag="pt")
                for c, w in enumerate(dch):
                    nc.tensor.transpose(
                        out=pt_ps[0:w, c * group: (c + 1) * group],
                        in_=pooled_sb[:, c * 128: c * 128 + w],
                        identity=ident[:group, :group],
                    )
                pt_sb = sbp.tile([128, nch * group], f32, tag="pt_sb")
                nc.vector.tensor_copy(pt_sb[:, 0:2 * group],
                                      pt_ps[:, 0:2 * group])
                nc.vector.tensor_copy(pt_sb[0:dch[2], 2 * group:3 * group],
                                      pt_ps[0:dch[2], 2 * group:3 * group])

                h_ps = ph_pool.tile([hid, group], f32, tag="h")
                for c, w in enumerate(dch):
                    nc.tensor.matmul(
                        h_ps[:],
                        lhsT=vwt_sb[0:w, c * hid: (c + 1) * hid],
                        rhs=pt_sb[0:w, c * group: (c + 1) * group],
                        start=(c == 0),
                        stop=(c == nch - 1),
                    )
                h_sb = sbp.tile([hid, group], f32, tag="h_sb")
                nc.scalar.activation(h_sb[:], h_ps[:], relu, bias=vb_sb[:, 0:1])

                l_ps = pl_pool.tile([nout, group], f32, tag="l")
                nc.tensor.matmul(l_ps[:], lhsT=wwt_sb[:], rhs=h_sb[:],
                                 start=True, stop=True)
                nc.vector.tensor_tensor(
                    out=out_sb[:, gg * group: (gg + 1) * group],
                    in0=l_ps[:],
                    in1=wb_sb[:, 0:1].to_broadcast([nout, group]),
                    op=mybir.AluOpType.add,
                )

        nc.sync.dma_start(t_out[:], out_sb[:])
    nc.finalize()
    return nc


def _pack_weights(V_w, V_b, W_w, W_b, dim=DIM, hid=HID, nout=OUT, seq=SEQ):
    nch = len(DCH)
    vwt = (np.asarray(V_w, np.float32).T
           / np.float32(seq * EMB_SCALE)).astype(np.float32)
    vwt_packed = np.zeros((128, nch * hid), np.float32)
    off = 0
    for c, w in enumerate(DCH):
        vwt_packed[0:w, c * hid: (c + 1) * hid] = vwt[off: off + w]
        off += w
    wwt = np.ascontiguousarray(np.asarray(W_w, np.float32).T)
    vb = np.asarray(V_b, np.float32).reshape(hid, 1)
    wb = np.asarray(W_b, np.float32).reshape(nout, 1)
    return vwt_packed, vb, wwt, wb


def _plan_and_pack(tokens, b_core=B_CORE, group=GROUP, seq=SEQ):
    """Bucket every core's tokens per (SG, bucket); compute cross-core
    budgets and block->group chunk lists; pack int16 index and per-chunk
    masked sentence-id tables per core."""
    import ml_dtypes

    n_cores = tokens.shape[0] // b_core
    flat = np.asarray(tokens, np.int64).reshape(
        n_cores, N_SG, SGG * group * seq)

    # slot metadata within a supergroup (group-major order)
    slot_group = np.repeat(np.arange(SGG), group * seq)
    slot_sent = np.tile(np.repeat(np.arange(group), seq), SGG)

    buck = flat >> 15
    counts = np.zeros((n_cores, N_SG, NB), np.int64)
    for b in range(NB):
        counts[:, :, b] = (buck == b).sum(axis=2)
    budgets = counts.max(axis=0)                     # [N_SG, NB]

    # per (core, sg): packed order (stable sort by bucket keeps group-major
    # order within each bucket)
    orders = np.argsort(buck, axis=2, kind="stable")
    sg_tok = np.take_along_axis(flat, orders, axis=2)
    sg_grp = slot_group[orders]
    sg_sen = slot_sent[orders]

    # chunk lists: per (sg, b, blk) the union over cores of groups present
    chunks = []
    for sg in range(N_SG):
        ch = []
        for b in range(NB):
            bud = int(budgets[sg, b])
            if bud == 0:
                continue
            nblk = _cdiv(bud, 128)
            tgt_sets = [set() for _ in range(nblk)]
            for c in range(n_cores):
                n = int(counts[c, sg, b])
                start = int(counts[c, sg, :b].sum())
                gb = sg_grp[c, sg, start:start + n]
                for i in range(nblk):
                    seg = gb[i * 128:(i + 1) * 128]
                    if seg.size:
                        for g in np.unique(seg):
                            tgt_sets[i].add(int(g))
            for i in range(nblk):
                if not tgt_sets[i]:
                    tgt_sets[i].add(0)      # all-pad block: harmless chunk
                for g in sorted(tgt_sets[i]):
                    ch.append((b, i, g))
        chunks.append(ch)
    plan = _Plan(budgets.tolist(), chunks)

    gidx = np.zeros((n_cores, 128, plan.icols_tot), np.int16)
    sent = np.full((n_cores, 128, plan.nchunk_tot), -1.0, ml_dtypes.bfloat16)
    for c in range(n_cores):
        for sg in range(N_SG):
            pos = 0
            for b in range(NB):
                n = int(counts[c, sg, b])
                bud = int(budgets[sg, b])
                if bud == 0:
                    continue
                loc = np.zeros(bud, np.int16)
                loc[:n] = (sg_tok[c, sg, pos:pos + n] & 32767).astype(np.int16)
                pos += n
                cols = _cdiv(bud, 16)
                w = np.zeros(cols * 16, np.int16)
                w[:bud] = loc
                io = plan.icol_off[sg][b]
                gidx[c, :, io:io + cols] = np.tile(
                    w.reshape(cols, 16).T, (8, 1))
            # per-chunk masked sentence ids
            coff = plan.chunk_off[sg]
            pos_b = np.concatenate([[0], np.cumsum(counts[c, sg])])
            for ci, (b, blk, tgt) in enumerate(plan.chunks[sg]):
                n = int(counts[c, sg, b])
                lo = blk * 128
                hi = min(lo + 128, n)
                if hi <= lo:
                    continue
                seg = slice(int(pos_b[b]) + lo, int(pos_b[b]) + hi)
                gb = sg_grp[c, sg, seg]
                se = sg_sen[c, sg, seg]
                col = np.full(128, -1.0, np.float32)
                m = gb == tgt
                col[0:hi - lo][m] = se[m]
                sent[c, :, coff + ci] = col.astype(ml_dtypes.bfloat16)
    return plan, gidx, sent


_STATE = {}


def kernel(tokens, emb, V_w, V_b, W_w, W_b, _trace=False):
    import ml_dtypes

    from concourse.bass_utils import run_bass_kernel_spmd

    tokens = np.asarray(tokens)
    emb = np.asarray(emb, np.float32)

    plan, gidx, sent = _plan_and_pack(tokens)
    vwt_packed, vb, wwt, wb = _pack_weights(V_w, V_b, W_w, W_b)

    embp = _STATE.get("embp")
    if embp is None or _STATE.get("embp_src") is not emb:
        embp = np.zeros((VOCAB, EPAD), ml_dtypes.float8_e4m3fn)
        embp[:, :DIM] = (emb * np.float32(EMB_SCALE)).astype(
            ml_dtypes.float8_e4m3fn)
        _STATE["embp"] = embp
        _STATE["embp_src"] = emb

    iota = np.broadcast_to(np.arange(GROUP, dtype=np.float32),
                           (128, GROUP)).astype(ml_dtypes.bfloat16).copy()
    ident = np.eye(128, dtype=np.float32)

    nc = None
    if _STATE.get("plan_key") == plan.key():
        nc = _STATE.get("nc")
    if nc is None:
        nc = _build_bass(plan)
        _STATE["nc"] = nc
        _STATE["plan_key"] = plan.key()

    in_maps = [
        {
            "gidx": np.ascontiguousarray(gidx[c]),
            "sent": np.ascontiguousarray(sent[c]),
            "iota": iota,
            "ident": ident,
            "embp": embp,
            "vwt": vwt_packed,
            "vb": vb,
            "wwt": wwt,
            "wb": wb,
        }
        for c in range(N_CORES)
    ]
    res = run_bass_kernel_spmd(nc, in_maps, core_ids=list(range(N_CORES)),
                               trace=_trace)
    _STATE["last_result"] = res

    logits = np.concatenate([r["out"].T for r in res.results], axis=0)

    # global log-softmax over the batch axis (LogSoftmax(dim=0))
    x = logits.astype(np.float64)
    m = x.max(axis=0, keepdims=True)
    lse = m + np.log(np.sum(np.exp(x - m), axis=0, keepdims=True))
    return (x - lse).astype(np.float32)


# revision 12
# speedup vs baseline: 1.5963x; 1.0412x over previous
"""DAN classifier (embedding gather + mean-pool + tiny MLP + batch log-softmax)
on 8 Trainium2 NeuronCores.

Sharding: data-parallel over the batch (sentence) dim — 2048 sentences/core.
The embedding table is replicated on every core as fp8-e4m3 scaled by 64
(padded to [400000, 512] so rows are 512 B — dma_gather needs elem_size
divisible by 256 B); the descale (and the 1/50 mean) folds into V_w. End to
end the fp8 quantization costs ~3e-5 rel err vs the 2e-2 budget: the
LogSoftmax(dim=0) output is dominated by -log(sum exp) over 16384 logits of
tiny variance, so per-logit absolute errors are ~1e-4.

Per-core device kernel, 4 supergroups (SG) of 512 sentences each:
  - The vocab is split into 13 buckets of 32768 rows so row indices fit the
    int16 index format of InstDMAGatherAnt. Tokens are bucketed per
    (SG, bucket) — 52 gathers of ~2100 rows each instead of 208 small ones:
    SWDGE descriptor generation is serialized on GPSIMD Q7 cores 0-1
    across ALL queues, so per-gather fixed cost (~1 us) must be amortized.
    Slot lists are packed (no per-group alignment), sorted by (bucket,
    group), padded to a cross-core budget with index-0 dummies (~3%).
  - Gathered rows land in SBUF tiles [128, nblk, 512] fp8: slot k ->
    partition k%128, block k//128.
  - Pooling: one matmul chunk per (128-slot block, target group). Blocks
    that straddle a group boundary (union over all 8 cores, computed on
    host) get one chunk per group they touch. The one-hot S for chunk c is
    S[p, c*128+s] = (sent_chunk[p, c] == s) where the host bakes group
    masking into sent_chunk: the local sentence id if slot p of the block
    belongs to chunk c's target group, else -1. One DVE is_equal per SG
    builds all chunks; S is stored fp8 (exact 0/1) to halve SBUF and let
    the PE run fp8 x fp8. Chunks accumulate into 4 live PSUM pooled banks
    (one per group) with start/stop on the first/last chunk per group.
    fp8 matmuls run single-pass (fp32 ran LOW_HIGH dual-pass) and keep the
    PE HAM-warm at 2.4 GHz.
  - MLP per group: PE transpose of pooled -> [300, 128]; matmuls against
    V_w.T/(SEQ*EMB_SCALE), ReLU+bias on ACT, W matmul, W_b add on DVE.
  - One DMA writes logits.T [2, 2048] to DRAM.

Host glue: shard/pack tokens, run SPMD on cores 0-7, concatenate the logit
slabs and apply the global log-softmax over the batch axis (16384x2 —
negligible next to the ~0.4GB on-device gather).
"""

import numpy as np

VOCAB, DIM, HID, OUT = 400000, 300, 32, 2
BATCH, SEQ = 16384, 50
N_CORES = 8
B_CORE = BATCH // N_CORES            # 2048 sentences per core
GROUP = 128                          # sentences per group
N_GROUPS = B_CORE // GROUP           # 16
SGG = 4                              # groups per supergroup
N_SG = N_GROUPS // SGG               # 4
EPAD = 512                           # padded fp8 row length (512 B)
EMB_SCALE = 64.0                     # fp8 table stores emb * 64
BUCKET = 32768                       # int16-addressable rows per bucket
NB = -(-VOCAB // BUCKET)             # 13
DCH = (128, 128, DIM - 256)          # contraction chunks over DIM
N_QUEUES = 4


def _cdiv(a, b):
    return -(-a // b)


class _Plan:
    """Per-(SG,bucket) budgets, chunk lists, and packed-layout offsets
    shared by the host packer and the device builder."""

    def __init__(self, budgets, chunks):
        self.budgets = budgets        # [N_SG][NB] ints
        self.chunks = chunks          # [N_SG] list of (b, blk, tgt)
        self.icol_off = []            # idx col offset per (sg,b)
        io = 0
        for sg in range(len(budgets)):
            row_i = []
            for b in range(NB):
                row_i.append(io)
                io += _cdiv(budgets[sg][b], 16)
            self.icol_off.append(row_i)
        self.icols_tot = io
        self.nblk = [[_cdiv(n, 128) for n in row] for row in budgets]
        self.max_nblk_b = max(max(r) for r in self.nblk)
        self.nchunk_sg = [len(c) for c in chunks]
        self.chunk_off = []
        co = 0
        for sg in range(len(chunks)):
            self.chunk_off.append(co)
            co += self.nchunk_sg[sg]
        self.nchunk_tot = co
        self.max_nchunk_sg = max(self.nchunk_sg)
        # first/last chunk index per (sg, tgt) for PSUM start/stop flags
        self.first_chunk = []
        self.last_chunk = []
        for sg in range(len(chunks)):
            f, l = {}, {}
            for ci, (b, blk, tgt) in enumerate(chunks[sg]):
                f.setdefault(tgt, ci)
                l[tgt] = ci
            self.first_chunk.append(f)
            self.last_chunk.append(l)

    def key(self):
        return (tuple(tuple(r) for r in self.budgets),
                tuple(tuple(c) for c in self.chunks))


def _build_bass(plan, vocab=VOCAB, dim=DIM, hid=HID, nout=OUT,
                b_core=B_CORE, group=GROUP, n_cores=N_CORES):
    from contextlib import ExitStack

    import concourse.tile as tile
    from concourse import bacc, mybir

    f32 = mybir.dt.float32
    bf16 = mybir.dt.bfloat16
    fp8 = mybir.dt.float8e4
    i16 = mybir.dt.int16
    dch = DCH
    nch = len(dch)

    nc = bacc.Bacc("TRN2", target_bir_lowering=False, debug=False,
                   enable_asserts=False, num_devices=n_cores,
                   num_swdge_queues=N_QUEUES)
    t_idx = nc.declare_dram_parameter("gidx", [128, plan.icols_tot], i16,
                                      isOutput=False)
    t_sent = nc.declare_dram_parameter("sent", [128, plan.nchunk_tot], bf16,
                                       isOutput=False)
    t_iota = nc.declare_dram_parameter("iota", [128, group], bf16,
                                       isOutput=False)
    t_ident = nc.declare_dram_parameter("ident", [128, 128], f32,
                                        isOutput=False)
    t_emb = nc.declare_dram_parameter("embp", [vocab, EPAD], fp8,
                                      isOutput=False)
    t_vwt = nc.declare_dram_parameter("vwt", [128, nch * hid], f32,
                                      isOutput=False)
    t_vb = nc.declare_dram_parameter("vb", [hid, 1], f32, isOutput=False)
    t_wwt = nc.declare_dram_parameter("wwt", [hid, nout], f32, isOutput=False)
    t_wb = nc.declare_dram_parameter("wb", [nout, 1], f32, isOutput=False)
    t_out = nc.declare_dram_parameter("out", [nout, b_core], f32,
                                      isOutput=True)

    relu = mybir.ActivationFunctionType.Relu
    is_eq = mybir.AluOpType.is_equal

    with ExitStack() as ctx:
        tc = ctx.enter_context(tile.TileContext(nc))
        consts = ctx.enter_context(tc.tile_pool(name="consts", bufs=1))
        gpool = ctx.enter_context(tc.tile_pool(name="gather", bufs=9))
        spool = ctx.enter_context(tc.tile_pool(name="smat", bufs=2))
        sbp = ctx.enter_context(tc.tile_pool(name="sbwork", bufs=2))
        pp_pool = ctx.enter_context(tc.tile_pool(name="ppool", bufs=1,
                                                 space="PSUM"))
        pt_pool = ctx.enter_context(tc.tile_pool(name="ptpool", bufs=2,
                                                 space="PSUM"))
        ph_pool = ctx.enter_context(tc.tile_pool(name="phpool", bufs=1,
                                                 space="PSUM"))
        pl_pool = ctx.enter_context(tc.tile_pool(name="plpool", bufs=1,
                                                 space="PSUM"))

        idx_sb = consts.tile([128, plan.icols_tot], i16)
        nc.sync.dma_start(idx_sb[:], t_idx[:])
        sent_sb = consts.tile([128, plan.nchunk_tot], bf16)
        nc.sync.dma_start(sent_sb[:], t_sent[:])
        iota_sb = consts.tile([128, group], bf16)
        nc.sync.dma_start(iota_sb[:], t_iota[:])
        ident = consts.tile([128, 128], f32)
        nc.sync.dma_start(ident[:], t_ident[:])
        vwt_sb = consts.tile([128, nch * hid], f32)
        nc.sync.dma_start(vwt_sb[:], t_vwt[:])
        vb_sb = consts.tile([hid, 1], f32)
        nc.sync.dma_start(vb_sb[:], t_vb[:])
        wwt_sb = consts.tile([hid, nout], f32)
        nc.sync.dma_start(wwt_sb[:], t_wwt[:])
        wb_sb = consts.tile([nout, 1], f32)
        nc.sync.dma_start(wb_sb[:], t_wb[:])
        out_sb = consts.tile([nout, b_core], f32)

        # Compute instructions carry at most ONE embedded sync wait after
        # codegen. Prime each engine's vector clock on every external
        # producer it will consume mid-loop, so steady-state instructions
        # need only the wait on their data tile.
        dumb_dve = consts.tile([hid, 1], f32)
        nc.vector.tensor_copy(dumb_dve[0:nout, :], wb_sb[:])
        nc.vector.tensor_copy(dumb_dve[:], sent_sb[0:hid, 0:1])
        nc.vector.tensor_copy(dumb_dve[:], iota_sb[0:hid, 0:1])
        dumb_act = consts.tile([hid, 1], f32)
        nc.scalar.copy(dumb_act[:], vb_sb[:])
        # prime PE on ACT-written SBUF (pooled_sb/pt_sb copies run on ACT)
        dumb_ps = pl_pool.tile([nout, group], f32, tag="l")
        nc.tensor.matmul(dumb_ps[0:1, 0:1], lhsT=ident[:, 0:1],
                         rhs=ident[:, 0:1], start=True, stop=True)
        nc.tensor.matmul(dumb_ps[0:1, 0:1], lhsT=dumb_act[:, 0:1],
                         rhs=dumb_act[:, 0:1], start=True, stop=True)
        nc.tensor.matmul(dumb_ps[0:1, 0:1], lhsT=vwt_sb[:, 0:1],
                         rhs=vwt_sb[:, 0:1], start=True, stop=True)
        nc.tensor.matmul(dumb_ps[0:1, 0:1], lhsT=wwt_sb[:, 0:1],
                         rhs=wwt_sb[:, 0:1], start=True, stop=True)

        N_SLICE = 4

        def alloc_s(sg):
            return spool.tile([128, plan.max_nchunk_sg * group], fp8,
                              tag="S", name="s_t")

        def build_s_slice(sg, s_t, j):
            """One-hot S for chunk slice j of supergroup sg:
            S[p, c*128+s] = (sent_chunk[p, c] == s). fp8 out (exact 0/1)
            runs the DVE at 1x, so split into N_SLICE ops to cut the
            latency in front of the next SG's first matmuls."""
            nchunk = plan.nchunk_sg[sg]
            c0 = j * nchunk // N_SLICE
            c1 = (j + 1) * nchunk // N_SLICE
            if c1 <= c0:
                return
            coff = plan.chunk_off[sg]
            in0 = sent_sb[:, coff + c0:coff + c1].to_broadcast(
                [128, c1 - c0, group])
            in1 = (iota_sb[:].rearrange("p (a c) -> p a c", a=1)
                   .to_broadcast([128, c1 - c0, group]))
            nc.vector.tensor_tensor(
                out=s_t[:, c0 * group:c1 * group].rearrange(
                    "p (c s) -> p c s", s=group),
                in0=in0, in1=in1, op=is_eq)

        def build_s(sg):
            s_t = alloc_s(sg)
            for j in range(N_SLICE):
                build_s_slice(sg, s_t, j)
            return s_t

        s_tiles = {0: build_s(0)}
        # prime PE on the DVE-built S
        nc.tensor.matmul(dumb_ps[0:1, 0:1], lhsT=s_tiles[0][:, 0:1],
                         rhs=s_tiles[0][:, 0:1], start=True, stop=True)

        gather_ct = 0
        for sg in range(N_SG):
            gtiles = []
            for b in range(NB):
                n = plan.budgets[sg][b]
                if n == 0:
                    gtiles.append(None)
                    continue
                nblk = plan.nblk[sg][b]
                gt = gpool.tile([128, plan.max_nblk_b * EPAD], fp8, tag="G")
                rows = min(BUCKET, vocab - b * BUCKET)
                io = plan.icol_off[sg][b]
                nc.gpsimd.dma_gather(
                    out_ap=gt[:, 0:nblk * EPAD].rearrange(
                        "p (c e) -> p c e", e=EPAD),
                    in_ap=t_emb[b * BUCKET: b * BUCKET + rows, :],
                    idxs_ap=idx_sb[:, io: io + _cdiv(n, 16)],
                    num_idxs=n,
                    num_idxs_reg=n,
                    elem_size=EPAD,
                    queue_num=gather_ct % N_QUEUES,
                    # >64 descriptors (1008 idxs) cannot fit one packet
                    single_packet=False,
                )
                gather_ct += 1
                gtiles.append(gt)

            s_sg = s_tiles.pop(sg)
            # S for sg+1 built now (DVE order: before this SG's pooled/pt
            # copies) so the next SG's matmuls carry no DVE wait.
            if sg + 1 < N_SG:
                s_tiles[sg + 1] = build_s(sg + 1)

            pooled = [pp_pool.tile([group, dim], f32, tag=f"pooled{g}",
                                   name=f"pooled{g}")
                      for g in range(SGG)]
            for ci, (b, blk, tgt) in enumerate(plan.chunks[sg]):
                k = min(128, plan.budgets[sg][b] - blk * 128)
                nc.tensor.matmul(
                    pooled[tgt][:],
                    lhsT=s_sg[0:k, ci * group: (ci + 1) * group],
                    rhs=gtiles[b][0:k, blk * EPAD: blk * EPAD + dim],
                    start=(ci == plan.first_chunk[sg][tgt]),
                    stop=(ci == plan.last_chunk[sg][tgt]),
                    skip_group_check=True,
                )

            for g in range(SGG):
                gg = sg * SGG + g
                pooled_sb = sbp.tile([group, dim], f32, tag="pooled_sb")
                nc.scalar.copy(pooled_sb[:], pooled[g][:])

                pt_ps = pt_pool.tile([128, nch * group], f32, tag="pt")
                for c, w in enumerate(dch):
                    nc.tensor.transpose(
                        out=pt_ps[0:w, c * group: (c + 1) * group],
                        in_=pooled_sb[:, c * 128: c * 128 + w],
                        identity=ident[:group, :group],
                    )
                pt_sb = sbp.tile([128, nch * group], f32, tag="pt_sb")
                nc.scalar.copy(pt_sb[:, 0:2 * group],
                               pt_ps[:, 0:2 * group])
                nc.scalar.copy(pt_sb[0:dch[2], 2 * group:3 * group],
                               pt_ps[0:dch[2], 2 * group:3 * group])

                h_ps = ph_pool.tile([hid, group], f32, tag="h")
                for c, w in enumerate(dch):
                    nc.tensor.matmul(
                        h_ps[:],
                        lhsT=vwt_sb[0:w, c * hid: (c + 1) * hid],
                        rhs=pt_sb[0:w, c * group: (c + 1) * group],
                        start=(c == 0),
                        stop=(c == nch - 1),
                    )
                h_sb = sbp.tile([hid, group], f32, tag="h_sb")
                nc.scalar.activation(h_sb[:], h_ps[:], relu, bias=vb_sb[:, 0:1])

                l_ps = pl_pool.tile([nout, group], f32, tag="l")
                nc.tensor.matmul(l_ps[:], lhsT=wwt_sb[:], rhs=h_sb[:],
                                 start=True, stop=True)
                nc.vector.tensor_tensor(
                    out=out_sb[:, gg * group: (gg + 1) * group],
                    in0=l_ps[:],
                    in1=wb_sb[:, 0:1].to_broadcast([nout, group]),
                    op=mybir.AluOpType.add,
                )

        nc.sync.dma_start(t_out[:], out_sb[:])
    nc.finalize()
    return nc


def _pack_weights(V_w, V_b, W_w, W_b, dim=DIM, hid=HID, nout=OUT, seq=SEQ):
    nch = len(DCH)
    vwt = (np.asarray(V_w, np.float32).T
           / np.float32(seq * EMB_SCALE)).astype(np.float32)
    vwt_packed = np.zeros((128, nch * hid), np.float32)
    off = 0
    for c, w in enumerate(DCH):
        vwt_packed[0:w, c * hid: (c + 1) * hid] = vwt[off: off + w]
        off += w
    wwt = np.ascontiguousarray(np.asarray(W_w, np.float32).T)
    vb = np.asarray(V_b, np.float32).reshape(hid, 1)
    wb = np.asarray(W_b, np.float32).reshape(nout, 1)
    return vwt_packed, vb, wwt, wb


def _plan_and_pack(tokens, b_core=B_CORE, group=GROUP, seq=SEQ):
    """Bucket every core's tokens per (SG, bucket); compute cross-core
    budgets and block->group chunk lists; pack int16 index and per-chunk
    masked sentence-id tables per core."""
    import ml_dtypes

    n_cores = tokens.shape[0] // b_core
    flat = np.asarray(tokens, np.int64).reshape(
        n_cores, N_SG, SGG * group * seq)

    # slot metadata within a supergroup (group-major order)
    slot_group = np.repeat(np.arange(SGG), group * seq)
    slot_sent = np.tile(np.repeat(np.arange(group), seq), SGG)

    buck = flat >> 15
    counts = np.zeros((n_cores, N_SG, NB), np.int64)
    for b in range(NB):
        counts[:, :, b] = (buck == b).sum(axis=2)
    budgets = counts.max(axis=0)                     # [N_SG, NB]

    # per (core, sg): packed order (stable sort by bucket keeps group-major
    # order within each bucket)
    orders = np.argsort(buck, axis=2, kind="stable")
    sg_tok = np.take_along_axis(flat, orders, axis=2)
    sg_grp = slot_group[orders]
    sg_sen = slot_sent[orders]

    # chunk lists: per (sg, b, blk) the union over cores of groups present
    chunks = []
    for sg in range(N_SG):
        ch = []
        for b in range(NB):
            bud = int(budgets[sg, b])
            if bud == 0:
                continue
            nblk = _cdiv(bud, 128)
            tgt_sets = [set() for _ in range(nblk)]
            for c in range(n_cores):
                n = int(counts[c, sg, b])
                start = int(counts[c, sg, :b].sum())
                gb = sg_grp[c, sg, start:start + n]
                for i in range(nblk):
                    seg = gb[i * 128:(i + 1) * 128]
                    if seg.size:
                        for g in np.unique(seg):
                            tgt_sets[i].add(int(g))
            for i in range(nblk):
                if not tgt_sets[i]:
                    tgt_sets[i].add(0)      # all-pad block: harmless chunk
                for g in sorted(tgt_sets[i]):
                    ch.append((b, i, g))
        chunks.append(ch)
    plan = _Plan(budgets.tolist(), chunks)

    gidx = np.zeros((n_cores, 128, plan.icols_tot), np.int16)
    sent = np.full((n_cores, 128, plan.nchunk_tot), -1.0, ml_dtypes.bfloat16)
    for c in range(n_cores):
        for sg in range(N_SG):
            pos = 0
            for b in range(NB):
                n = int(counts[c, sg, b])
                bud = int(budgets[sg, b])
                if bud == 0:
                    continue
                loc = np.zeros(bud, np.int16)
                loc[:n] = (sg_tok[c, sg, pos:pos + n] & 32767).astype(np.int16)
                pos += n
                cols = _cdiv(bud, 16)
                w = np.zeros(cols * 16, np.int16)
                w[:bud] = loc
                io = plan.icol_off[sg][b]
                gidx[c, :, io:io + cols] = np.tile(
                    w.reshape(cols, 16).T, (8, 1))
            # per-chunk masked sentence ids
            coff = plan.chunk_off[sg]
            pos_b = np.concatenate([[0], np.cumsum(counts[c, sg])])
            for ci, (b, blk, tgt) in enumerate(plan.chunks[sg]):
                n = int(counts[c, sg, b])
                lo = blk * 128
                hi = min(lo + 128, n)
                if hi <= lo:
                    continue
                seg = slice(int(pos_b[b]) + lo, int(pos_b[b]) + hi)
                gb = sg_grp[c, sg, seg]
                se = sg_sen[c, sg, seg]
                col = np.full(128, -1.0, np.float32)
                m = gb == tgt
                col[0:hi - lo][m] = se[m]
                sent[c, :, coff + ci] = col.astype(ml_dtypes.bfloat16)
    return plan, gidx, sent


_STATE = {}


def kernel(tokens, emb, V_w, V_b, W_w, W_b, _trace=False):
    import ml_dtypes

    from concourse.bass_utils import run_bass_kernel_spmd

    tokens = np.asarray(tokens)
    emb = np.asarray(emb, np.float32)

    plan, gidx, sent = _plan_and_pack(tokens)
    vwt_packed, vb, wwt, wb = _pack_weights(V_w, V_b, W_w, W_b)

    embp = _STATE.get("embp")
    if embp is None or _STATE.get("embp_src") is not emb:
        embp = np.zeros((VOCAB, EPAD), ml_dtypes.float8_e4m3fn)
        embp[:, :DIM] = (emb * np.float32(EMB_SCALE)).astype(
            ml_dtypes.float8_e4m3fn)
        _STATE["embp"] = embp
        _STATE["embp_src"] = emb

    iota = np.broadcast_to(np.arange(GROUP, dtype=np.float32),
                           (128, GROUP)).astype(ml_dtypes.bfloat16).copy()
    ident = np.eye(128, dtype=np.float32)

    nc = None
    if _STATE.get("plan_key") == plan.key():
        nc = _STATE.get("nc")
    if nc is None:
        nc = _build_bass(plan)
        _STATE["nc"] = nc
        _STATE["plan_key"] = plan.key()

    in_maps = [
        {
            "gidx": np.ascontiguousarray(gidx[c]),
            "sent": np.ascontiguousarray(sent[c]),
            "iota": iota,
            "ident": ident,
            "embp": embp,
            "vwt": vwt_packed,
            "vb": vb,
            "wwt": wwt,
            "wb": wb,
        }
        for c in range(N_CORES)
    ]
    res = run_bass_kernel_spmd(nc, in_maps, core_ids=list(range(N_CORES)),
                               trace=_trace)
    _STATE["last_result"] = res

    logits = np.concatenate([r["out"].T for r in res.results], axis=0)

    # global log-softmax over the batch axis (LogSoftmax(dim=0))
    x = logits.astype(np.float64)
    m = x.max(axis=0, keepdims=True)
    lse = m + np.log(np.sum(np.exp(x - m), axis=0, keepdims=True))
    return (x - lse).astype(np.float32)
